# revision 28
# baseline (speedup 1.0000x reference)
"""Trainium2 Bass kernel for nn_ExcitationModule (YIN pitch -> harmonic synthesis).

Sharding: B=4 rows x 2 halves of T=131072 across 8 cores (pure data parallel;
the phase cumsum carry for the second half of each row is recomputed locally
from the first-half pitch_mult, so no collectives are needed).

Per core layout: 65536 samples as [128 partitions x 512], t = p*512 + q.
Pipeline per core:
  1. YIN on the full row (128 frames on partitions): autocorrelation via a
     2048-point DFT as bf16 PE matmuls, difference function, CMNDF,
     threshold/argmax logic.  (identical to the previous version)
  2. phase = cumsum(2*pi*f0/FS) via per-partition scan + PE lower-triangular
     prefix matmul + carry; phi wrapped to [-0.5, 0.5] turns; c = 0.5/theta
     (Nyquist cutoff in harmonic units, clamped to finite).
  3. synthesis in harmonic-major layout driven by the PE ("selector" matmuls):
     for each p (one 512-sample block), a one-hot column-scaled selector
     lhsT produces y[h, q] = h*phi[p, q] and hc[h, q] = h*c[p, q] tiles
     straight into PSUM (f32r, 1 cycle/row).  DVE rounds n = (y+C)-C
     (C = 1.5*2^23 forces integer quantization), the PE subtracts n back
     via a -identity matmul accumulated into the same PSUM (range
     reduction without a second vector pass), ACT evaluates Sin(2*pi*f)
     on 1024-wide tiles, DVE/Pool apply the Nyquist mask in one
     scalar_tensor_tensor ((hc > h^2) * s, since h*c > h^2 <=> h < c),
     and the PE contracts the 150 harmonics with amplitude-selector
     matmuls accumulated into a persistent PSUM signal tile.
     Harmonics 129..150 ride in 26 packed tiles (5 sample-blocks each).
"""

import numpy as np
import ml_dtypes
from contextlib import ExitStack

FS = 44100.0
NH = 150
TAU_MIN = 110
FRAME = 1260
B, T = 4, 131072
NF = 128
NFFT = 2048
NBINS = 1024
HALF = 65536
P, Q = 128, 512
BIGF = 1.0e6
TWO_PI = 2.0 * np.pi
L519 = 519
CMAG = float(1.5 * 2 ** 23)   # round-to-int magic constant
NHI = 22                      # harmonics 129..150
NGHI = 26                     # ceil(128 / 5) packed hi tiles
NUNITS = 128 + NGHI           # 154 synthesis units

_BF16 = ml_dtypes.bfloat16
_cache = {}

# scheduling knobs (engine assignment patterns per group), tuned from traces
ROUND_PAT = "A"       # round engine: A=ACT two-phase, D=DVE (no Pool: PSUM)
MASKS_PAT = "D"        # mask-apply engine (SBUF bf16): P=Pool, D=DVE


def _host_consts():
    j = np.arange(1280)
    k = np.arange(NBINS)
    w = np.zeros((1280, 2 * NBINS), dtype=np.float64)
    ang = 2.0 * np.pi * np.outer(j[:FRAME], k) / NFFT
    w[:FRAME, :NBINS] = np.cos(ang)
    w[:FRAME, NBINS:] = np.sin(ang)
    wdft = w.astype(_BF16)

    tau = np.arange(630)
    v = np.cos(2.0 * np.pi * np.outer(k, tau) / NFFT)
    v[1:, :] *= 2.0
    vidft = v.astype(_BF16)
    vny = ((-1.0) ** tau).reshape(1, 630).astype(_BF16)

    alts = np.zeros((1280, 1), dtype=np.float64)
    alts[:FRAME, 0] = (-1.0) ** j[:FRAME]
    altsign = alts.astype(_BF16)

    ident = np.eye(128, dtype=_BF16)
    lt = (np.arange(128)[:, None] < np.arange(128)[None, :]).astype(np.float32)
    ones_row = np.ones((1, 128), dtype=np.float32)

    msel = []
    for h in (0, 1):
        m = np.zeros((128, 128), dtype=np.float32)
        m[h * 64 + np.arange(128) // 2, np.arange(128)] = 1.0
        msel.append(m)
    msel0 = np.zeros((128, 128), dtype=np.float32)
    msel0[np.arange(128) // 2, np.arange(128)] = 1.0

    taus = np.arange(1, 630).astype(np.float32).reshape(1, 629)
    iota519 = np.arange(L519).astype(np.float32).reshape(1, L519)

    # --- synthesis selectors ---
    hv = np.arange(1, 129, dtype=np.float32)
    # windowed 64-wide selectors, duplicated on both partition halves:
    # sel64[i, 128*o + j] = (j+1) * ((i % 64) == o)
    sel64 = np.zeros((128, 64 * 128), dtype=np.float32)
    for o in range(64):
        sel64[o, 128 * o:128 * (o + 1)] = hv
        sel64[o + 64, 128 * o:128 * (o + 1)] = hv
    # hi-harmonic packed selectors: col r = 22*ii + kk -> harmonic 129+kk of
    # sample-block p = 5*g + ii
    selhi = np.zeros((128, NGHI * 128), dtype=np.float32)
    pattern_hi = np.zeros((128, NGHI * 128), dtype=np.float32)
    for g in range(NGHI):
        for ii in range(5):
            p = 5 * g + ii
            if p >= 128:
                break
            for kk in range(NHI):
                r = 22 * ii + kk
                selhi[p, 128 * g + r] = 129.0 + kk
                pattern_hi[r, 128 * g + p] = 1.0
    negI = (-np.eye(128)).astype(_BF16)
    h2lo = (hv ** 2).reshape(128, 1).astype(np.float32)
    h2hi = np.full((128, 1), 1.0e18, dtype=np.float32)
    rr = np.arange(110)
    h2hi[rr, 0] = (129.0 + (rr % NHI)) ** 2

    return dict(wdft=wdft, vidft=vidft, vny=vny, altsign=altsign, ident=ident,
                lt=lt, ones_row=ones_row, msel=msel, msel0=msel0,
                taus=taus, iota519=iota519,
                sel64=sel64, selhi=selhi, pattern_hi=pattern_hi.astype(_BF16),
                negI=negI, h2lo=h2lo, h2hi=h2hi)


def _ap(t, off_delta, free_dims):
    import concourse.bass as bass
    return bass.AP(t.tensor, t.offset + off_delta, [t.ap[0]] + free_dims)


def _build_nc():
    import concourse.bass as bass
    import concourse.bacc as bacc
    import concourse.mybir as mybir
    import concourse.tile as tile

    f32 = mybir.dt.float32
    f32r = mybir.dt.float32r
    bf16 = mybir.dt.bfloat16
    i32 = mybir.dt.int32
    AX = mybir.AxisListType.X
    OP = mybir.AluOpType
    ACTF = mybir.ActivationFunctionType

    nc = bacc.Bacc(trn_type="TRN2")

    audio = nc.dram_tensor("audio", [T], f32, kind="ExternalInput")
    pm_d = nc.dram_tensor("pm", [P, Q], f32, kind="ExternalInput")
    pmc_d = nc.dram_tensor("pmc", [P, Q], f32, kind="ExternalInput")
    msel_d = nc.dram_tensor("msel", [128, 128], f32, kind="ExternalInput")
    msel0_d = nc.dram_tensor("msel0", [128, 128], f32, kind="ExternalInput")
    wdft_d = nc.dram_tensor("wdft", [1280, 2 * NBINS], bf16, kind="ExternalInput")
    vidft_d = nc.dram_tensor("vidft", [NBINS, 630], bf16, kind="ExternalInput")
    vny_d = nc.dram_tensor("vny", [1, 630], bf16, kind="ExternalInput")
    alts_d = nc.dram_tensor("altsign", [1280, 1], bf16, kind="ExternalInput")
    ident_d = nc.dram_tensor("ident", [128, 128], bf16, kind="ExternalInput")
    lt_d = nc.dram_tensor("ltmask", [128, 128], f32, kind="ExternalInput")
    ones_d = nc.dram_tensor("ones_row", [1, 128], f32, kind="ExternalInput")
    ratio_d = nc.dram_tensor("ratio_in", [1, 1], f32, kind="ExternalInput")
    taus_d = nc.dram_tensor("taus", [1, 629], f32, kind="ExternalInput")
    iota_d = nc.dram_tensor("iota519", [1, L519], f32, kind="ExternalInput")
    sel64_d = nc.dram_tensor("sel64", [128, 64 * 128], f32r, kind="ExternalInput")
    selhi_d = nc.dram_tensor("selhi", [128, NGHI * 128], f32r, kind="ExternalInput")
    pathi_d = nc.dram_tensor("pattern_hi", [128, NGHI * 128], bf16, kind="ExternalInput")
    negI_d = nc.dram_tensor("negI", [128, 128], bf16, kind="ExternalInput")
    h2lo_d = nc.dram_tensor("h2lo", [128, 1], f32, kind="ExternalInput")
    h2hi_d = nc.dram_tensor("h2hi", [128, 1], f32, kind="ExternalInput")
    amplo_d = nc.dram_tensor("amp_lo", [128, 1], f32, kind="ExternalInput")
    amphi_d = nc.dram_tensor("amp_hi", [128, 1], f32, kind="ExternalInput")
    out_d = nc.dram_tensor("sig_out", [HALF], f32, kind="ExternalOutput")

    def bc(dram, n, parts=128):
        return bass.AP(dram, 0, [[0, parts], [1, n]])

    with ExitStack() as ctx:
        tc = ctx.enter_context(tile.TileContext(nc))
        const = ctx.enter_context(tc.tile_pool(name="const", bufs=1))
        syn_keep = ctx.enter_context(tc.tile_pool(name="syn_keep", bufs=1))

        # ---- synthesis constants (DMA first so they arrive early) ----
        sel64_t = const.tile([128, 64 * 128], f32r)
        nc.sync.dma_start(out=sel64_t, in_=sel64_d.ap())
        selhi_t = const.tile([128, NGHI * 128], f32r)
        nc.sync.dma_start(out=selhi_t, in_=selhi_d.ap())
        patctx = ExitStack()
        patpool = patctx.enter_context(tc.tile_pool(name="patpool", bufs=1))
        pathi_t = patpool.tile([128, NGHI * 128], bf16)
        nc.sync.dma_start(out=pathi_t, in_=pathi_d.ap())
        negI_t = const.tile([128, 128], bf16)
        nc.sync.dma_start(out=negI_t, in_=negI_d.ap())
        h2lo_t = const.tile([128, 1], f32)
        nc.sync.dma_start(out=h2lo_t, in_=h2lo_d.ap())
        h2hi_t = const.tile([128, 1], f32)
        nc.sync.dma_start(out=h2hi_t, in_=h2hi_d.ap())
        amplo_raw = const.tile([128, 1], f32)
        nc.sync.dma_start(out=amplo_raw, in_=amplo_d.ap())
        amphi_raw = const.tile([128, 1], f32)
        nc.sync.dma_start(out=amphi_raw, in_=amphi_d.ap())
        ratio_t = const.tile([128, 1], f32)
        nc.sync.dma_start(out=ratio_t, in_=bc(ratio_d, 1))

        # ---- YIN constants ----
        taus_t = const.tile([128, 629], f32)
        nc.sync.dma_start(out=taus_t, in_=bc(taus_d, 629))
        iota_t = const.tile([128, L519], f32)
        nc.sync.dma_start(out=iota_t, in_=bc(iota_d, L519))
        msel_t = const.tile([128, 128], f32)
        nc.sync.dma_start(out=msel_t, in_=msel_d.ap())
        msel0_t = const.tile([128, 128], f32)
        nc.sync.dma_start(out=msel0_t, in_=msel0_d.ap())
        lt_t = const.tile([128, 128], f32)
        nc.sync.dma_start(out=lt_t, in_=lt_d.ap())
        ones_t = const.tile([1, 128], f32)
        nc.sync.dma_start(out=ones_t, in_=ones_d.ap())
        ident_t = const.tile([128, 128], bf16)
        nc.sync.dma_start(out=ident_t, in_=ident_d.ap())
        vny_t = const.tile([1, 630], bf16)
        nc.sync.dma_start(out=vny_t, in_=vny_d.ap())
        alts_t = const.tile([128, 10], bf16)
        nc.sync.dma_start(out=alts_t, in_=bass.AP(alts_d, 0, [[1, 128], [128, 10]]))
        pm_t = syn_keep.tile([P, Q], f32)
        nc.sync.dma_start(out=pm_t, in_=pm_d.ap())
        pmc_t = const.tile([P, Q], f32)
        nc.sync.dma_start(out=pmc_t, in_=pmc_d.ap())

        # runtime amplitude columns (amp * ratio)
        amplo_t = const.tile([128, 1], f32)
        nc.vector.tensor_scalar_mul(amplo_t, amplo_raw, ratio_t[:, 0:1])
        amphi_t = const.tile([128, 1], f32)
        nc.vector.tensor_scalar_mul(amphi_t, amphi_raw, ratio_t[:, 0:1])
        # amp-selector tiles for the harmonic reduce:
        # lo unit u: 64-col window, col (u % 64) = amp_lo (reduce writes a
        # 64-row half of sig); hi unit g: full 128-col (5 rows may straddle
        # the 64-partition boundary), pattern * amp_hi
        ampsel_lo = const.tile([128, 128 * 64], bf16)
        nc.vector.memset(ampsel_lo, 0.0)
        nc.vector.tensor_copy(_ap(ampsel_lo, 0, [[65, 64]]),
                              _ap(amplo_t, 0, [[0, 64]]))
        nc.vector.tensor_copy(_ap(ampsel_lo, 64 * 64, [[65, 64]]),
                              _ap(amplo_t, 0, [[0, 64]]))
        ampsel_hi = const.tile([128, NGHI * 128], bf16)
        nc.vector.tensor_scalar_mul(ampsel_hi, pathi_t, amphi_t[:, 0:1])
        patctx.close()   # release the pattern tile before YIN needs SBUF

        pitchS = const.tile([128, 1], f32)
        phic = syn_keep.tile([P, 2 * Q], f32r)   # [phi | c] matmul rhs

        # ================= YIN =================
        with ExitStack() as yctx:
            ypool = yctx.enter_context(tc.tile_pool(name="yin", bufs=1))
            psT = yctx.enter_context(tc.tile_pool(name="psT", bufs=2, space="PSUM"))

            f_t = ypool.tile([128, FRAME], f32)
            nc.sync.dma_start(out=f_t, in_=bass.AP(audio, 0, [[1021, 128], [1, FRAME]]))
            fb = ypool.tile([128, 1280], bf16)
            nc.vector.memset(_ap(fb, FRAME, [[1, 1280 - FRAME]]), 0.0)
            nc.vector.tensor_copy(fb[:, 0:FRAME], f_t)

            ftb_all = ypool.tile([128, 1280], bf16)
            ftb = [ftb_all[:, 128 * c:128 * (c + 1)] for c in range(10)]
            for c in range(10):
                tp = psT.tile([128, 128], bf16, tag="tp")
                nc.tensor.transpose(tp, fb[:, 128 * c:128 * (c + 1)], ident_t)
                nc.vector.tensor_copy(ftb[c], tp)

            wt_all = ypool.tile([128, 10 * 2 * NBINS], bf16)
            wt = [wt_all[:, 2 * NBINS * c:2 * NBINS * (c + 1)] for c in range(10)]
            for c in range(10):
                nc.sync.dma_start(out=wt[c], in_=wdft_d.ap()[128 * c:128 * (c + 1), :])

            with ExitStack() as sctx:
                psS = sctx.enter_context(tc.tile_pool(name="psS", bufs=1, space="PSUM"))
                psNy = sctx.enter_context(tc.tile_pool(name="psNy", bufs=1, space="PSUM"))
                s_ps = psS.tile([128, 2 * NBINS], f32)
                for kc in range(4):
                    for c in range(10):
                        nc.tensor.matmul(s_ps[:, 512 * kc:512 * (kc + 1)],
                                         lhsT=ftb[c], rhs=wt[c][:, 512 * kc:512 * (kc + 1)],
                                         start=(c == 0), stop=(c == 9))
                sny_ps = psNy.tile([1, 128], f32)
                for c in range(10):
                    nc.tensor.matmul(sny_ps, lhsT=alts_t[:, c:c + 1],
                                     rhs=ftb[c], start=(c == 0), stop=(c == 9))

                sq_scale = float(1.0 / np.sqrt(NFFT))
                t1 = ypool.tile([128, NBINS], f32)
                nc.scalar.activation(t1, s_ps[:, 0:NBINS], ACTF.Square, scale=sq_scale)
                t2 = ypool.tile([128, NBINS], f32)
                nc.scalar.activation(t2, s_ps[:, NBINS:2 * NBINS], ACTF.Square, scale=sq_scale)
                pb = ypool.tile([128, NBINS], bf16)
                nc.vector.tensor_add(pb, t1, t2)
                pnyT = ypool.tile([1, 128], bf16)
                nc.scalar.activation(pnyT, sny_ps, ACTF.Square, scale=sq_scale)

            ptb = ypool.tile([128, NBINS], bf16)
            for c in range(8):
                tp = psT.tile([128, 128], bf16, tag="tp")
                nc.tensor.transpose(tp, pb[:, 128 * c:128 * (c + 1)], ident_t)
                nc.vector.tensor_copy(ptb[:, 128 * c:128 * (c + 1)], tp)

            vt_all = ypool.tile([128, 8 * 630], bf16)
            vt = [vt_all[:, 630 * c:630 * (c + 1)] for c in range(8)]
            for c in range(8):
                nc.sync.dma_start(out=vt[c], in_=vidft_d.ap()[128 * c:128 * (c + 1), :])

            with ExitStack() as cctx:
                psC = cctx.enter_context(tc.tile_pool(name="psC", bufs=1, space="PSUM"))
                corr_ps = psC.tile([128, 1024], f32)
                for (a, b) in ((0, 512), (512, 630)):
                    for c in range(8):
                        nc.tensor.matmul(corr_ps[:, a:b],
                                         lhsT=ptb[:, 128 * c:128 * (c + 1)],
                                         rhs=vt[c][:, a:b], start=(c == 0), stop=False)
                    nc.tensor.matmul(corr_ps[:, a:b], lhsT=pnyT,
                                     rhs=vny_t[:, a:b], start=False, stop=True)
                corr_t = ypool.tile([128, 630], f32)
                nc.vector.tensor_copy(corr_t, corr_ps[:, 0:630])

            f2 = ypool.tile([128, FRAME], f32)
            nc.scalar.square(f2, f_t)
            e_t = ypool.tile([128, FRAME], f32)
            nc.vector.tensor_tensor_scan(e_t, f2, f2, 0.0, OP.add, OP.bypass)

            e_rev = _ap(e_t, 1258, [[-1, 629]])
            e_lo = _ap(e_t, 0, [[1, 629]])
            d_t = ypool.tile([128, 629], f32)
            nc.vector.tensor_sub(d_t, e_rev, e_lo)
            nc.vector.scalar_tensor_tensor(d_t, corr_t[:, 1:630], -2.0, d_t,
                                           OP.mult, OP.add)
            nc.vector.tensor_scalar_add(d_t, d_t, e_t[:, 1259:1260])

            dsum = ypool.tile([128, 629], f32)
            nc.vector.tensor_tensor_scan(dsum, d_t, d_t, 0.0, OP.add, OP.bypass)
            nc.vector.tensor_scalar_max(dsum, dsum, 1e-5)
            numer = ypool.tile([128, 629], f32)
            nc.vector.tensor_mul(numer, d_t, taus_t)
            sden = ypool.tile([128, 629], f32)
            nc.vector.tensor_scalar_mul(sden, dsum, 0.1)
            ns = numer[:, TAU_MIN:629]
            ds_den = dsum[:, TAU_MIN:629]

            below = ypool.tile([128, L519], f32)
            nc.vector.tensor_tensor(below, ns, sden[:, TAU_MIN:629], OP.is_lt)
            cand = ypool.tile([128, L519], f32)
            nc.vector.scalar_tensor_tensor(cand, below, -BIGF, iota_t, OP.mult, OP.add)
            mi = ypool.tile([128, 1], f32)
            nc.vector.tensor_reduce(mi, cand, AX, OP.min)
            fbv = ypool.tile([128, 1], f32)
            nc.vector.tensor_scalar_add(fbv, mi, BIGF)
            m1 = ypool.tile([128, 1], f32)
            nc.vector.tensor_scalar(m1, fbv, 1.0, None, OP.is_ge)
            m2 = ypool.tile([128, 1], f32)
            nc.vector.tensor_scalar(m2, fbv, 630.0, None, OP.is_le)
            nc.vector.tensor_mul(m1, m1, m2)
            fb_t = ypool.tile([128, 1], f32)
            nc.vector.scalar_tensor_tensor(fb_t, fbv, -630.0, m1, OP.add, OP.mult)
            nc.vector.tensor_scalar_add(fb_t, fb_t, 630.0)

            beyond = ypool.tile([128, L519], f32)
            nc.vector.tensor_scalar(beyond, iota_t, fb_t[:, 0:1], None, OP.is_ge)

            slope = ypool.tile([128, L519], f32)
            nc.vector.memset(slope, 1.0)
            xm1 = ypool.tile([128, L519 - 1], f32)
            nc.vector.tensor_mul(xm1, ns[:, 1:L519], ds_den[:, 0:L519 - 1])
            xm0 = ypool.tile([128, L519 - 1], f32)
            nc.vector.tensor_mul(xm0, ns[:, 0:L519 - 1], ds_den[:, 1:L519])
            nc.vector.tensor_tensor(slope[:, 0:L519 - 1], xm1, xm0, OP.is_ge)

            nc.vector.tensor_mul(beyond, beyond, slope)
            nc.vector.scalar_tensor_tensor(cand, beyond, -BIGF, iota_t, OP.mult, OP.add)
            nc.vector.tensor_reduce(mi, cand, AX, OP.min)
            tauv = ypool.tile([128, 1], f32)
            nc.vector.tensor_scalar_add(tauv, mi, BIGF)
            m3 = ypool.tile([128, 1], f32)
            nc.vector.tensor_scalar(m3, tauv, 630.0, None, OP.is_le)
            nc.vector.tensor_mul(tauv, tauv, m3)
            m4 = ypool.tile([128, 1], f32)
            nc.vector.tensor_scalar(m4, tauv, 1.0, None, OP.is_ge)
            ptau = ypool.tile([128, 1], f32)
            nc.vector.tensor_scalar_add(ptau, tauv, float(TAU_MIN + 1))
            rp = ypool.tile([128, 1], f32)
            nc.vector.reciprocal(rp, ptau)
            nc.vector.tensor_mul(pitchS, rp, m4)   # pitch/FS per frame (turns)

        # ============ phase & cutoff ============
        with ExitStack() as pctx:
            ppool = pctx.enter_context(tc.tile_pool(name="ph", bufs=1))
            psSm = pctx.enter_context(tc.tile_pool(name="psSm", bufs=1, space="PSUM"))

            pp_ps = psSm.tile([128, 1], f32)
            nc.tensor.matmul(pp_ps, lhsT=msel_t, rhs=pitchS, start=True, stop=True)
            ppartS = ppool.tile([128, 1], f32)
            nc.vector.tensor_copy(ppartS, pp_ps)

            p0_ps = psSm.tile([128, 1], f32)
            nc.tensor.matmul(p0_ps, lhsT=msel0_t, rhs=pitchS, start=True, stop=True)
            p0S = ppool.tile([128, 1], f32)
            nc.vector.tensor_copy(p0S, p0_ps)

            pmsum = ppool.tile([128, 1], f32)
            nc.vector.reduce_sum(pmsum, pmc_t, axis=AX)
            car_ps = psSm.tile([1, 1], f32)
            nc.tensor.matmul(car_ps, lhsT=p0S, rhs=pmsum, start=True, stop=True)
            car_sb = ppool.tile([1, 1], f32)
            nc.vector.tensor_copy(car_sb, car_ps)

            theta = ppool.tile([P, Q], f32)
            nc.vector.tensor_scalar_mul(theta, pm_t, ppartS[:, 0:1])
            sc_t = ppool.tile([P, Q], f32)
            nc.vector.tensor_tensor_scan(sc_t, theta, theta, 0.0, OP.add, OP.bypass)

            offs_ps = psSm.tile([128, 1], f32)
            nc.tensor.matmul(offs_ps, lhsT=lt_t, rhs=sc_t[:, Q - 1:Q],
                             start=True, stop=False)
            nc.tensor.matmul(offs_ps, lhsT=ones_t, rhs=car_sb,
                             start=False, stop=True)
            offs = ppool.tile([128, 1], f32)
            nc.vector.tensor_copy(offs, offs_ps)
            phi_t = ppool.tile([P, Q], f32)
            nc.vector.tensor_scalar_add(phi_t, sc_t, offs[:, 0:1])
            # wrap phi to [-0.5, 0.5] turns: phi -= round(phi)
            nphi = ppool.tile([P, Q], f32)
            nc.vector.tensor_scalar(nphi, phi_t, CMAG, -CMAG, OP.add, OP.add)
            nc.vector.scalar_tensor_tensor(phi_t, nphi, -1.0, phi_t,
                                           OP.mult, OP.add)
            nc.vector.tensor_copy(phic[:, 0:Q], phi_t)

            # c = 0.5/theta clamped finite (theta >= 0)
            thc = ppool.tile([P, Q], f32)
            nc.vector.tensor_scalar_max(thc, theta, 1e-12)
            c_t = ppool.tile([P, Q], f32)
            nc.vector.reciprocal(c_t, thc)
            nc.vector.tensor_scalar_mul(c_t, c_t, 0.5)
            nc.vector.tensor_copy(phic[:, Q:2 * Q], c_t)

        # ============ synthesis ============
        psY = ctx.enter_context(tc.tile_pool(name="psY", bufs=2, space="PSUM"))
        psHC = ctx.enter_context(tc.tile_pool(name="psHC", bufs=1, space="PSUM"))
        psSig = ctx.enter_context(tc.tile_pool(name="psSig", bufs=1, space="PSUM"))
        psSig2 = ctx.enter_context(tc.tile_pool(name="psSig2", bufs=1, space="PSUM"))
        npool = ctx.enter_context(tc.tile_pool(name="npool", bufs=2))
        spool = ctx.enter_context(tc.tile_pool(name="spool", bufs=4))
        mpool = ctx.enter_context(tc.tile_pool(name="mpool", bufs=4))
        upool = ctx.enter_context(tc.tile_pool(name="upool", bufs=2))
        biasC_t = const.tile([128, 1], f32)
        nc.vector.memset(biasC_t, CMAG)

        # two signal banks, one accumulation group each: rows 0:64 / 64:128
        sigA_ps = psSig.tile([128, Q], f32)
        sigB_ps = psSig2.tile([128, Q], f32)

        def unit_mm(dst_ap, u, which, start, stop):
            # which: 0 -> phi half (y), 1 -> c half (hc)
            rhs_cols = (0, Q) if which == 0 else (Q, 2 * Q)
            if u < 128:
                base = 64 * (u // 64)
                o = u % 64
                nc.tensor.matmul(
                    dst_ap,
                    lhsT=sel64_t[base:base + 64, 128 * o:128 * (o + 1)],
                    rhs=phic[base:base + 64, rhs_cols[0]:rhs_cols[1]],
                    start=start, stop=stop)
            else:
                g = u - 128
                nc.tensor.matmul(
                    dst_ap,
                    lhsT=selhi_t[:, 128 * g:128 * (g + 1)],
                    rhs=phic[:, rhs_cols[0]:rhs_cols[1]],
                    start=start, stop=stop)

        groups = [(2 * i, 2 * i + 1) for i in range(NUNITS // 2)]
        ng = len(groups)
        st = {}

        def emit_reduce(gi):
            uA, uB = groups[gi]
            s_t = st[gi]["s"]
            for half, u in ((0, uA), (1, uB)):
                rhs = s_t[:, half * Q:(half + 1) * Q]
                if u < 128:
                    if u < 64:
                        nc.tensor.matmul(sigA_ps[0:64, :],
                                         lhsT=_ap(ampsel_lo, 64 * u, [[1, 64]]),
                                         rhs=rhs, start=(u == 0), stop=False)
                    else:
                        nc.tensor.matmul(sigB_ps[64:128, :],
                                         lhsT=_ap(ampsel_lo, 64 * u, [[1, 64]]),
                                         rhs=rhs, start=(u == 64), stop=False)
                else:
                    g = u - 128
                    if g <= 12:   # target rows 5g..5g+4 intersect [0, 64)
                        nc.tensor.matmul(sigA_ps[0:64, :],
                                         lhsT=_ap(ampsel_hi, 128 * g, [[1, 64]]),
                                         rhs=rhs, start=False, stop=(g == 12))
                    if g >= 12:   # target rows intersect [64, 128)
                        nc.tensor.matmul(sigB_ps[64:128, :],
                                         lhsT=_ap(ampsel_hi, 128 * g + 64, [[1, 64]]),
                                         rhs=rhs, start=False, stop=(g == NGHI - 1))

        # Software pipeline over groups, lags chosen so every engine's
        # per-iteration work depends only on >=1-iteration-old results
        # (except documented short same-iteration handoffs):
        #   it:   HCmm(it), Ymm(it)                     [PE]
        #   it+1: round(it) | m_extract(it)             [DVE/Pool alternating]
        #   it+2: nsub(it)  [PE];  sin(it)              [ACT]
        #   it+3: maskS(it) = s*m                       [D/P]
        #   it+4: reduce(it)                            [PE]
        for it in range(ng + 4):
            g2 = it - 2
            if 0 <= g2 < ng:    # nsub first: sin(g2) waits only on this
                Y2, n2 = st[g2]["Y"], st[g2]["n"]
                nc.tensor.matmul(Y2[:, 0:Q], lhsT=negI_t, rhs=n2[:, 0:Q],
                                 start=False, stop=True)
                nc.tensor.matmul(Y2[:, Q:2 * Q], lhsT=negI_t, rhs=n2[:, Q:2 * Q],
                                 start=False, stop=True)
                s_t = spool.tile([128, 2 * Q], bf16, tag="s")
                nc.scalar.activation(s_t, Y2, ACTF.Sin, scale=float(TWO_PI))
                st[g2]["s"] = s_t
            g4 = it - 4
            if 0 <= g4 < ng:
                emit_reduce(g4)
                del st[g4]
            if it < ng:
                # same-selector matmuls adjacent so Ldweights are elided
                uA, uB = groups[it]
                Y = psY.tile([128, 2 * Q], f32, tag="Y")
                HCa = psHC.tile([128, Q], f32, tag="HC")
                HCb = psHC.tile([128, Q], f32, tag="HC")
                unit_mm(Y[:, 0:Q], uA, 0, start=True, stop=False)
                unit_mm(HCa, uA, 1, start=True, stop=True)
                unit_mm(Y[:, Q:2 * Q], uB, 0, start=True, stop=False)
                unit_mm(HCb, uB, 1, start=True, stop=True)
                st[it] = {"Y": Y, "HC": (HCa, HCb)}
            g1 = it - 1
            if 0 <= g1 < ng:
                Y1, HC1 = st[g1]["Y"], st[g1]["HC"]
                n_t = npool.tile([128, 2 * Q], bf16, tag="n")
                rmode = ROUND_PAT[g1 % len(ROUND_PAT)]
                if rmode == "A":
                    # two-phase: ACT adds C (psum->sbuf f32, integer-quantized),
                    # DVE subtracts C at sbuf 2x rate
                    u_t = upool.tile([128, 2 * Q], f32, tag="u")
                    nc.scalar.activation(u_t, Y1, ACTF.Identity, bias=biasC_t[:, 0:1])
                    nc.vector.tensor_scalar_add(n_t, u_t, -CMAG)
                elif rmode == "D":
                    nc.vector.tensor_scalar(n_t, Y1, CMAG, -CMAG, OP.add, OP.add)
                else:
                    nc.vector.tensor_scalar(n_t, Y1, CMAG, -CMAG, OP.add, OP.add)
                st[g1]["n"] = n_t
                h2col = h2hi_t if groups[g1][0] >= 128 else h2lo_t
                m_t = mpool.tile([128, 2 * Q], bf16, tag="m")
                for half in (0, 1):
                    nc.vector.tensor_scalar(m_t[:, half * Q:(half + 1) * Q],
                                            HC1[half], h2col[:, 0:1], None, OP.is_gt)
                st[g1]["m"] = m_t
            g3 = it - 3
            if 0 <= g3 < ng:
                mseng = nc.gpsimd if MASKS_PAT[g3 % len(MASKS_PAT)] == "P" else nc.vector
                mseng.tensor_mul(st[g3]["s"], st[g3]["s"], st[g3]["m"])

        sig = syn_keep.tile([P, Q], f32)
        nc.vector.tensor_copy(sig[0:64, :], sigA_ps[0:64, :])
        nc.vector.tensor_copy(sig[64:128, :], sigB_ps[64:128, :])
        nc.sync.dma_start(out=bass.AP(out_d, 0, [[Q, P], [1, Q]]), in_=sig)

    nc.finalize()
    return nc


def kernel(audio, pitch_mult, amplitudes, ratio):
    from concourse.bass_utils import run_bass_kernel_spmd

    audio = np.ascontiguousarray(np.asarray(audio, dtype=np.float32))
    pitch_mult = np.ascontiguousarray(np.asarray(pitch_mult, dtype=np.float32))
    amplitudes = np.ascontiguousarray(np.asarray(amplitudes, dtype=np.float32))
    ratio = np.ascontiguousarray(np.asarray(ratio, dtype=np.float32))

    if "nc" not in _cache:
        _cache["nc"] = _build_nc()
        _cache["consts"] = _host_consts()
    nc = _cache["nc"]
    cc = _cache["consts"]

    amp_lo = amplitudes[0:128].reshape(128, 1).copy()
    amp_hi = np.zeros((128, 1), dtype=np.float32)
    rr = np.arange(110)
    amp_hi[rr, 0] = amplitudes[128 + (rr % NHI)]

    in_maps = []
    for core in range(8):
        r, h = core // 2, core % 2
        pm = pitch_mult[r, h * HALF:(h + 1) * HALF].reshape(P, Q).copy()
        if h == 1:
            pmc = pitch_mult[r, 0:HALF].reshape(P, Q).copy()
        else:
            pmc = np.zeros((P, Q), dtype=np.float32)
        in_maps.append({
            "audio": audio[r].copy(),
            "pm": pm,
            "pmc": pmc,
            "msel": cc["msel"][h],
            "msel0": cc["msel0"],
            "wdft": cc["wdft"],
            "vidft": cc["vidft"],
            "vny": cc["vny"],
            "altsign": cc["altsign"],
            "ident": cc["ident"],
            "ltmask": cc["lt"],
            "ones_row": cc["ones_row"],
            "ratio_in": ratio.reshape(1, 1),
            "taus": cc["taus"],
            "iota519": cc["iota519"],
            "sel64": cc["sel64"],
            "selhi": cc["selhi"],
            "pattern_hi": cc["pattern_hi"],
            "negI": cc["negI"],
            "h2lo": cc["h2lo"],
            "h2hi": cc["h2hi"],
            "amp_lo": amp_lo,
            "amp_hi": amp_hi,
        })

    res = run_bass_kernel_spmd(nc, in_maps, core_ids=list(range(8)))
    out = np.zeros((B, T), dtype=np.float32)
    for core in range(8):
        r, h = core // 2, core % 2
        out[r, h * HALF:(h + 1) * HALF] = res.results[core]["sig_out"]
    return out
